# revision 19
# baseline (speedup 1.0000x reference)
"""Trainium2 Bass kernel for nn_Attention (qkv+BN -> biased softmax attention -> gelu -> proj+BN).

Sharding: data-parallel over batch B=128 across 8 NeuronCores (16 batches each).
BatchNorm (training-mode) statistics are all-reduced across cores (tiny collectives).

Per-core layout strategy:
  - x transposed on HOST to xT [C, rows] bf16 (no on-chip transposes).
  - qkv computed as [h_dim, row] with HOST-permuted Wqkv so the 1536 h-dims land
    as [q(8x32) | k(8x32) | v(8x128)] -> chunks 0-1 q, 2-3 k, 4-11 v (one v chunk per head).
  - v additionally produced ROW-major per batch (vb tiles [m, dv]) by small
    matmuls from xT, replacing the per-tile DMA transposes of v.
  - softmax(s+bias) = exp(s)*exp(bias) / rowsum: exp(bias) gathered on host (replicated),
    multiplied on DVE/GpSimd; rowsums via ones-matmul on PE; divide deferred past AV.
  - attention loop is batch-outer / head-inner; AV output overwrites that head's
    v slice in SBUF.
  - divide+gelu+proj+BN2 in a final phase (single activation-table switch to gelu).
"""
import os
import numpy as np
import ml_dtypes

import concourse.bass as bass
import concourse.tile as tile
from concourse import bacc, mybir
from concourse.bass_utils import run_bass_kernel_spmd

NCORES = int(os.environ.get("KERN_NCORES", "8"))
DBG_STOP = os.environ.get("KERN_STOP", "")   # "A" or "B" to stop early
TRACE_SIM = os.environ.get("KERN_TRACE_SIM", "") == "1"
B, N, C = 128, 320, 256
NH, DK, DV = 8, 32, 128
H = NH * (2 * DK + DV)       # 1536
DH = NH * DV                 # 1024
BL = B // 8                  # 16 batches/core (fixed shard size)
R = BL * N                   # 5120 rows/core
NT = B * N                   # 40960 global rows
EPS = 1e-5
SCALE = DK ** -0.5
FP = mybir.dt.float32
BF = mybir.dt.bfloat16

NHC = H // 128               # 12 h-chunks
NRB = R // 512               # 10 row blocks of 512
MCS = [128, 128, 64]         # chunking of N=320
AF = mybir.ActivationFunctionType
OP = mybir.AluOpType


def build_program():
    nc = bacc.Bacc("TRN2", target_bir_lowering=False, debug=False,
                   enable_asserts=False, num_devices=NCORES)
    xT_d = nc.dram_tensor("xT", [C, R], BF, kind="ExternalInput").ap()
    xb_d = nc.dram_tensor("xb", [R, C + 1], BF, kind="ExternalInput").ap()
    wqkvT_d = nc.dram_tensor("wqkvT", [C, H], BF, kind="ExternalInput").ap()
    wprojT_d = nc.dram_tensor("wprojT", [DH, C], BF, kind="ExternalInput").ap()
    eb4_d = nc.dram_tensor("eb4", [2, 3, 128, 4 * N], BF, kind="ExternalInput").ap()
    g1_d = nc.dram_tensor("g1c", [128, NHC], FP, kind="ExternalInput").ap()
    b1_d = nc.dram_tensor("b1c", [128, NHC], FP, kind="ExternalInput").ap()
    g2_d = nc.dram_tensor("g2c", [128, 2], FP, kind="ExternalInput").ap()
    b2_d = nc.dram_tensor("b2c", [128, 2], FP, kind="ExternalInput").ap()
    yT_d = nc.dram_tensor("yT", [C, R], FP, kind="ExternalOutput").ap()

    with tile.TileContext(nc, trace_sim=TRACE_SIM) as tc:
        with tc.tile_pool(name="const", bufs=1) as constp, \
             tc.tile_pool(name="qkv", bufs=1) as qkvp, \
             tc.tile_pool(name="dram", bufs=1, space="DRAM") as dramp, \
             tc.tile_pool(name="stat", bufs=1) as statp:

            # ---- constants ----
            wprojT_sb = constp.tile([128, NH * C], BF)     # 8 d-chunks side by side
            for dc in range(NH):
                nc.sync.dma_start(wprojT_sb[:, dc * C:(dc + 1) * C],
                                  wprojT_d[dc * 128:(dc + 1) * 128, :])
            g1_sb = constp.tile([128, NHC], FP)
            b1_sb = constp.tile([128, NHC], FP)
            g2_sb = constp.tile([128, 2], FP)
            b2_sb = constp.tile([128, 2], FP)
            nc.sync.dma_start(g1_sb[:], g1_d[:])
            nc.sync.dma_start(b1_sb[:], b1_d[:])
            nc.sync.dma_start(g2_sb[:], g2_d[:])
            nc.sync.dma_start(b2_sb[:], b2_d[:])
            ones_c = constp.tile([128, 1], BF)             # ones column (bf16 matmuls)
            nc.vector.memset(ones_c[:], 1.0)
            ones_rb = constp.tile([128, 128], BF)          # bf16 ones (outers)
            nc.vector.memset(ones_rb[:], 1.0)

            # ---- persistent big buffers ----
            qkv_sb = [qkvp.tile([128, R], BF, tag=f"qkv{i}", name=f"qkv{i}")
                      for i in range(NHC)]

            # xT / Wqkv columns live through Phase A + B (v-row matmuls), not C
            with tc.tile_pool(name="xa", bufs=1) as xp:
                xT_sb = [xp.tile([128, R], BF, tag=f"xT{cc}", name=f"xT{cc}")
                         for cc in range(2)]
                wq_sb = [xp.tile([128, H], BF, tag=f"wq{cc}", name=f"wq{cc}")
                         for cc in range(2)]
                for cc in range(2):
                    nc.sync.dma_start(xT_sb[cc][:], xT_d[cc * 128:(cc + 1) * 128, :])
                    nc.sync.dma_start(wq_sb[cc][:], wqkvT_d[cc * 128:(cc + 1) * 128, :])

                # ========== Phase A: q,k matmul + Gram-matrix BN1 stats ==========
                # sum_r qkv[h,r]  = Wqkv @ (sum_r x_r)        (xsum = G[:,256])
                # sum_r qkv[h,r]^2 = w_h^T (x^T x) w_h = sum_c WT[c,h]*(G@WT)[c,h]
                stats = statp.tile([128, 2 * NHC], FP)
                with tc.tile_pool(name="pa", bufs=2, space="PSUM") as pap:
                    # q,k chunks only (hc 0-3); v is produced row-major in Phase B
                    for rb in range(NRB):
                        for hc in range(4):
                            pq = pap.tile([128, 512], FP, tag="pq")
                            for cc in range(2):
                                nc.tensor.matmul(
                                    pq[:],
                                    wq_sb[cc][:, hc * 128:(hc + 1) * 128],
                                    xT_sb[cc][:, rb * 512:(rb + 1) * 512],
                                    start=(cc == 0), stop=(cc == 1))
                            nc.vector.tensor_copy(
                                qkv_sb[hc][:, rb * 512:(rb + 1) * 512], pq[:])
                    # Gram G = [x|1]^T [x|1], accumulated over 40 row chunks
                    Gsb = [statp.tile([128, C + 1], BF, tag=f"Gsb{_g}", name=f"Gsb{_g}")
                           for _g in range(2)]
                    with tc.tile_pool(name="gx", bufs=8) as gxp, \
                         tc.tile_pool(name="pg", bufs=1, space="PSUM") as pgp:
                        G_ps = [pgp.tile([128, C + 1], FP, tag=f"G{gg}", name=f"G{gg}")
                                for gg in range(2)]
                        for rc in range(40):
                            xbt = gxp.tile([128, C + 1], BF, tag="xb")
                            nc.sync.dma_start(xbt[:], xb_d[rc * 128:(rc + 1) * 128, :])
                            for gg in range(2):
                                nc.tensor.matmul(
                                    G_ps[gg][:], xbt[:, gg * 128:(gg + 1) * 128],
                                    xbt[:], start=(rc == 0), stop=(rc == 39))
                        for gg in range(2):
                            nc.vector.tensor_copy(Gsb[gg][:], G_ps[gg][:])
                    # P1 = G @ WT ; prod = P1 .* WT ; column-sum -> sumsq row.
                    # sums row from xsum (= Gsb[:,256]) @ WT. Rows land at
                    # partitions 0 (sums) / 32 (sumsq) of rows_ps[nc3].
                    rows_sb0 = statp.tile([1, H], BF, name="rows_sb0")  # per-h sums
                    rows_sb1 = statp.tile([1, H], BF, name="rows_sb1")  # per-h sumsq
                    with tc.tile_pool(name="pp1", bufs=2, space="PSUM") as pp1, \
                         tc.tile_pool(name="prw", bufs=3, space="PSUM") as prw, \
                         tc.tile_pool(name="pss", bufs=1, space="PSUM") as pssp, \
                         tc.tile_pool(name="sc", bufs=4) as scp:
                        rows_ps = [prw.tile([128, 512], FP, tag="rows", name=f"rows{_r}")
                                   for _r in range(3)]
                        for nc3 in range(3):
                            for gg in range(2):
                                p1 = pp1.tile([128, 512], FP, tag="p1")
                                for gp in range(2):
                                    nc.tensor.matmul(
                                        p1[:], Gsb[gp][:, gg * 128:(gg + 1) * 128],
                                        wq_sb[gp][:, nc3 * 512:(nc3 + 1) * 512],
                                        start=(gp == 0), stop=(gp == 1))
                                prod = scp.tile([128, 512], BF, tag="prod")
                                nc.vector.tensor_tensor(
                                    prod[:], p1[:],
                                    wq_sb[gg][:, nc3 * 512:(nc3 + 1) * 512], OP.mult)
                                nc.tensor.matmul(
                                    rows_ps[nc3][32:33, :], ones_c[:, 0:1], prod[:],
                                    start=(gg == 0), stop=(gg == 1),
                                    tile_position=(0, 32))
                            # separate accumulation group, AFTER sumsq completes
                            for gg in range(2):
                                nc.tensor.matmul(
                                    rows_ps[nc3][0:1, :], Gsb[gg][:, 256:257],
                                    wq_sb[gg][:, nc3 * 512:(nc3 + 1) * 512],
                                    start=(gg == 0), stop=(gg == 1),
                                    tile_position=(0, 0))
                            nc.vector.tensor_copy(
                                rows_sb0[0:1, nc3 * 512:(nc3 + 1) * 512],
                                rows_ps[nc3][0:1, :])
                            nc.vector.tensor_copy(
                                rows_sb1[0:1, nc3 * 512:(nc3 + 1) * 512],
                                rows_ps[nc3][32:33, :])
                        # transpose rows -> per-partition stat columns [128, 24]
                        stats_ps = pssp.tile([128, 2 * NHC], FP, tag="stp")
                        for hc in range(NHC):
                            nc.tensor.matmul(
                                stats_ps[:, hc:hc + 1],
                                rows_sb0[0:1, hc * 128:(hc + 1) * 128],
                                ones_c[0:1, 0:1])
                            nc.tensor.matmul(
                                stats_ps[:, NHC + hc:NHC + hc + 1],
                                rows_sb1[0:1, hc * 128:(hc + 1) * 128],
                                ones_c[0:1, 0:1])
                        nc.vector.tensor_copy(stats[:], stats_ps[:])
                bounce_i = dramp.tile([128, 2 * NHC], FP, tag="b1i")
                bounce_o = dramp.tile([128, 2 * NHC], FP, tag="b1o")
                nc.sync.dma_start(bounce_i[:], stats[:])
                nc.gpsimd.collective_compute(
                    "AllReduce", OP.add,
                    replica_groups=[list(range(NCORES))],
                    ins=[bounce_i.opt()], outs=[bounce_o.opt()])
                statsg = statp.tile([128, 2 * NHC], FP)
                nc.sync.dma_start(statsg[:], bounce_o[:])

                mean1 = statp.tile([128, NHC], FP)
                var1 = statp.tile([128, NHC], FP)
                tmp1 = statp.tile([128, NHC], FP)
                alpha1 = statp.tile([128, NHC], FP)
                beta1 = statp.tile([128, NHC], FP)
                nc.vector.tensor_scalar(mean1[:], statsg[:, 0:NHC], 1.0 / NT, None,
                                        OP.mult)
                nc.vector.tensor_scalar(var1[:], statsg[:, NHC:2 * NHC], 1.0 / NT,
                                        None, OP.mult)
                nc.vector.tensor_tensor(tmp1[:], mean1[:], mean1[:], OP.mult)
                nc.vector.tensor_tensor(var1[:], var1[:], tmp1[:], OP.subtract)
                nc.vector.tensor_scalar(var1[:], var1[:], EPS, None, OP.add)
                nc.scalar.activation(tmp1[:], var1[:], AF.Ln)
                nc.scalar.activation(var1[:], tmp1[:], AF.Exp, scale=-0.5)   # rstd
                nc.vector.tensor_tensor(alpha1[:], g1_sb[:], var1[:], OP.mult)
                nc.vector.tensor_tensor(beta1[:], mean1[:], alpha1[:], OP.mult)
                nc.vector.tensor_tensor(beta1[:], b1_sb[:], beta1[:], OP.subtract)
                for hc in range(4):                   # q,k only; v folded into gelu
                    nc.vector.tensor_scalar(
                        qkv_sb[hc][:], qkv_sb[hc][:],
                        alpha1[:, hc:hc + 1], beta1[:, hc:hc + 1], OP.mult, OP.add)

                if DBG_STOP not in ("A",):
                    # ========== Phase B: attention, batch-major ==========
                    # Per (g, mc): 4 heads' QK land concurrently (row groups
                    # 0/32/64/96) in one 4-bank PSUM tile; one batched exp
                    # (FD=1280) + one bias multiply cover all 4 heads.
                    with tc.tile_pool(name="ps4", bufs=2, space="PSUM") as ps4p, \
                         tc.tile_pool(name="pb", bufs=2, space="PSUM") as pbp, \
                         tc.tile_pool(name="prx", bufs=1, space="PSUM") as prxp, \
                         tc.tile_pool(name="pv", bufs=1, space="PSUM") as pvp, \
                         tc.tile_pool(name="eb2", bufs=4) as ep, \
                         tc.tile_pool(name="rrp", bufs=2) as rrp, \
                         tc.tile_pool(name="sc2", bufs=4) as scp2, \
                         tc.tile_pool(name="ebp", bufs=1) as ebp, \
                         tc.tile_pool(name="vb", bufs=6) as vbp:
                        eb4_sb = []                        # [g][mc] -> [128, 4*320]
                        for g in range(2):
                            row = []
                            for mc in range(3):
                                t = ebp.tile([128, 4 * N], BF, tag=f"eb{g}_{mc}",
                                             name=f"eb{g}_{mc}")
                                nc.sync.dma_start(t[:], eb4_d[g, mc])
                                row.append(t)
                            eb4_sb.append(row)
                        for b in range(BL):
                            # v row-major tiles for this batch: vb[g][mc] holds
                            # 4 heads' dv side by side at partitions [mb, mb+ms)
                            vb = [[None] * 3 for _ in range(2)]
                            for g in range(2):
                                for mc in range(3):
                                    ms = MCS[mc]
                                    mb = 64 if mc == 2 else 0
                                    vp_ps = pvp.tile([128, 512], FP, tag="vp")
                                    for cc in range(2):
                                        nc.tensor.matmul(
                                            vp_ps[mb:mb + ms, :],
                                            xT_sb[cc][:, b * N + 128 * mc:
                                                       b * N + 128 * mc + ms],
                                            wq_sb[cc][:, 512 + g * 512:
                                                      1024 + g * 512],
                                            start=(cc == 0), stop=(cc == 1),
                                            tile_position=(0, mb))
                                    vt = vbp.tile([128, 512], BF, tag="vb")
                                    nc.vector.tensor_copy(vt[mb:mb + ms, :],
                                                          vp_ps[mb:mb + ms, :])
                                    vb[g][mc] = vt
                            r_recip = []
                            for g in range(2):
                                et4s = []
                                for mc in range(3):
                                    ms = MCS[mc]
                                    mb = 64 if mc == 2 else 0
                                    et4 = ep.tile([128, 4 * N], BF, tag="et")
                                    for hp in range(2):
                                        s2 = ps4p.tile([128, 1024], FP, tag="s2",
                                                       name="s2")
                                        for hi in range(2):
                                            hh = 2 * hp + hi
                                            qr = 32 * hh
                                            nc.tensor.matmul(
                                                s2[mb:mb + ms,
                                                   hi * 512:hi * 512 + N],
                                                qkv_sb[2 + g][qr:qr + 32,
                                                              b * N + 128 * mc:
                                                              b * N + 128 * mc + ms],
                                                qkv_sb[g][qr:qr + 32,
                                                          b * N:(b + 1) * N],
                                                tile_position=(qr, mb))
                                        sin = s2[mb:mb + ms, :].rearrange(
                                            "p (h n) -> p h n", h=2)[:, :, 0:N]
                                        eout = et4[mb:mb + ms,
                                                   hp * 2 * N:(hp + 1) * 2 * N
                                                   ].rearrange(
                                            "p (h n) -> p h n", h=2)
                                        nc.scalar.activation(eout, sin, AF.Exp,
                                                             scale=SCALE)
                                    nc.vector.tensor_tensor(
                                        et4[mb:mb + ms, :], et4[mb:mb + ms, :],
                                        eb4_sb[g][mc][mb:mb + ms, :], OP.mult)
                                    et4s.append(et4)
                                rp = prxp.tile([128, N], FP, tag="rx", name="rp")
                                for hh in range(4):
                                    rrow = 32 * hh
                                    for mc in range(3):
                                        ms = MCS[mc]
                                        mb = 64 if mc == 2 else 0
                                        nc.tensor.matmul(
                                            rp[rrow:rrow + 1, :],
                                            ones_c[mb:mb + ms, 0:1],
                                            et4s[mc][mb:mb + ms,
                                                     hh * N:(hh + 1) * N],
                                            start=(mc == 0), stop=(mc == 2),
                                            tile_position=(mb, rrow))
                                for hh in range(4):
                                    h = 4 * g + hh
                                    av = pbp.tile([128, N], FP, tag="av")
                                    for mc in range(3):
                                        ms = MCS[mc]
                                        mb = 64 if mc == 2 else 0
                                        nc.tensor.matmul(
                                            av[:],
                                            vb[g][mc][mb:mb + ms,
                                                      hh * 128:hh * 128 + 128],
                                            et4s[mc][mb:mb + ms,
                                                     hh * N:(hh + 1) * N],
                                            start=(mc == 0), stop=(mc == 2),
                                            tile_position=(mb, 0))
                                    if hh % 2:
                                        nc.scalar.copy(
                                            qkv_sb[4 + h][:, b * N:(b + 1) * N],
                                            av[:])
                                    else:
                                        nc.vector.tensor_copy(
                                            qkv_sb[4 + h][:, b * N:(b + 1) * N],
                                            av[:])
                                rr = rrp.tile([128, N], FP, tag="rr", name="rr")
                                nc.vector.reciprocal_approx_fast(rr[:], rp[:])
                                r_recip.append(rr)
                            # softmax divide for this batch
                            rrb16 = []
                            for g in range(2):
                                rb16 = scp2.tile([128, N], BF, tag="rb16",
                                                 name="rb16")
                                nc.vector.tensor_copy(rb16[:], r_recip[g][:])
                                rrb16.append(rb16)
                            for h in range(NH):
                                rr = rrb16[h // 4]
                                rbr = 32 * (h % 4)
                                rb_ps = prxp.tile([128, N], FP, tag="rx",
                                                  name="rbps")
                                nc.tensor.matmul(
                                    rb_ps[:], ones_rb[rbr:rbr + 1, :],
                                    rr[rbr:rbr + 1, :], tile_position=(rbr, 0))
                                nc.vector.tensor_tensor(
                                    qkv_sb[4 + h][:, b * N:(b + 1) * N],
                                    qkv_sb[4 + h][:, b * N:(b + 1) * N],
                                    rb_ps[:], OP.mult)

            if DBG_STOP not in ("A", "B"):
                # ========== Phase C: gelu, transposed proj (yT), BN2 ==========
                # yT[c, r] = sum_dh WprojT[dh, c] * gelu(av)[dh, r]; BN2 stats
                # and affine become per-partition (c on partitions).
                with tc.tile_pool(name="ppy", bufs=4, space="PSUM") as ppy, \
                     tc.tile_pool(name="yb", bufs=1) as yp, \
                     tc.tile_pool(name="sc3", bufs=4) as scp3:
                    for h in range(NH):
                        nc.scalar.activation(qkv_sb[4 + h][:], qkv_sb[4 + h][:],
                                             AF.Gelu,
                                             scale=alpha1[:, 4 + h:5 + h],
                                             bias=beta1[:, 4 + h:5 + h])
                    yT_sb = [yp.tile([128, R], BF, tag=f"yT{c2}", name=f"yT{c2}")
                             for c2 in range(2)]
                    sq_sb = yp.tile([128, R], BF, name="sq_sb")   # square scratch
                    s2sum = statp.tile([128, 2 * NRB], FP)
                    s2sq = statp.tile([128, 2], FP)
                    for rb in range(NRB):
                        for c2 in range(2):
                            py = ppy.tile([128, 512], FP, tag="py")
                            for h in range(NH):
                                nc.tensor.matmul(
                                    py[:],
                                    wprojT_sb[:, h * C + c2 * 128:
                                              h * C + c2 * 128 + 128],
                                    qkv_sb[4 + h][:, rb * 512:(rb + 1) * 512],
                                    start=(h == 0), stop=(h == NH - 1))
                            nc.vector.tensor_scalar(
                                yT_sb[c2][:, rb * 512:(rb + 1) * 512], py[:],
                                1.0, 0.0, OP.mult, OP.add,
                                accum_out=s2sum[:, c2 * NRB + rb:
                                                c2 * NRB + rb + 1])
                    for c2 in range(2):
                        nc.scalar.activation(sq_sb[:], yT_sb[c2][:], AF.Square,
                                             accum_out=s2sq[:, c2:c2 + 1])
                    st2 = statp.tile([128, 4], FP)
                    for c2 in range(2):
                        nc.vector.tensor_reduce(
                            st2[:, c2:c2 + 1], s2sum[:, c2 * NRB:(c2 + 1) * NRB],
                            mybir.AxisListType.X, OP.add)
                    nc.vector.tensor_copy(st2[:, 2:4], s2sq[:])
                    b2i = dramp.tile([128, 4], FP, tag="b2i")
                    b2o = dramp.tile([128, 4], FP, tag="b2o")
                    nc.sync.dma_start(b2i[:], st2[:])
                    nc.gpsimd.collective_compute(
                        "AllReduce", OP.add,
                        replica_groups=[list(range(NCORES))],
                        ins=[b2i.opt()], outs=[b2o.opt()])
                    st2g = statp.tile([128, 4], FP)
                    nc.sync.dma_start(st2g[:], b2o[:])

                    mean2 = statp.tile([128, 2], FP)
                    var2 = statp.tile([128, 2], FP)
                    tmp2 = statp.tile([128, 2], FP)
                    alpha2 = statp.tile([128, 2], FP)
                    beta2 = statp.tile([128, 2], FP)
                    nc.vector.tensor_scalar(mean2[:], st2g[:, 0:2], 1.0 / NT, None,
                                            OP.mult)
                    nc.vector.tensor_scalar(var2[:], st2g[:, 2:4], 1.0 / NT,
                                            None, OP.mult)
                    nc.vector.tensor_tensor(tmp2[:], mean2[:], mean2[:], OP.mult)
                    nc.vector.tensor_tensor(var2[:], var2[:], tmp2[:], OP.subtract)
                    nc.vector.tensor_scalar(var2[:], var2[:], EPS, None, OP.add)
                    nc.scalar.activation(tmp2[:], var2[:], AF.Ln)
                    nc.scalar.activation(var2[:], tmp2[:], AF.Exp, scale=-0.5)
                    nc.vector.tensor_tensor(alpha2[:], g2_sb[:], var2[:], OP.mult)
                    nc.vector.tensor_tensor(beta2[:], mean2[:], alpha2[:], OP.mult)
                    nc.vector.tensor_tensor(beta2[:], b2_sb[:], beta2[:],
                                            OP.subtract)

                    for rb in range(NRB):
                        for c2 in range(2):
                            yo = scp3.tile([128, 512], FP, tag="yo")
                            nc.vector.tensor_scalar(
                                yo[:], yT_sb[c2][:, rb * 512:(rb + 1) * 512],
                                alpha2[:, c2:c2 + 1], beta2[:, c2:c2 + 1],
                                OP.mult, OP.add)
                            nc.sync.dma_start(
                                yT_d[c2 * 128:(c2 + 1) * 128,
                                     rb * 512:(rb + 1) * 512], yo[:])
            if DBG_STOP in ("A", "B"):
                dsrc = qkv_sb[0] if DBG_STOP == "A" else qkv_sb[4]
                for i in range(20):
                    dq = statp.tile([128, C], FP, tag="dq", name="dq", bufs=2)
                    nc.vector.tensor_copy(dq[:], dsrc[:, i * C:(i + 1) * C])
                    nc.sync.dma_start(yT_d[0:128, i * C:(i + 1) * C], dq[:])

    nc.compile()
    return nc


_PROG = None


def _get_prog():
    global _PROG
    if _PROG is None:
        _PROG = build_program()
    return _PROG


def _host_prep(x, Wqkv, g1, b1, ab, Wproj, g2, b2, idxs):
    perm = np.empty(H, dtype=np.int64)
    for h in range(NH):
        base = h * (2 * DK + DV)
        perm[DK * h: DK * (h + 1)] = np.arange(base, base + DK)
        perm[NH * DK + DK * h: NH * DK + DK * (h + 1)] = \
            np.arange(base + DK, base + 2 * DK)
        perm[2 * NH * DK + DV * h: 2 * NH * DK + DV * (h + 1)] = \
            np.arange(base + 2 * DK, base + 2 * DK + DV)
    x = np.asarray(x, dtype=np.float32)
    Wqkv = np.asarray(Wqkv, dtype=np.float32)
    wqkvT = np.ascontiguousarray(Wqkv[perm, :].T).astype(ml_dtypes.bfloat16)
    g1c = np.ascontiguousarray(np.asarray(g1, np.float32)[perm].reshape(NHC, 128).T)
    b1c = np.ascontiguousarray(np.asarray(b1, np.float32)[perm].reshape(NHC, 128).T)
    wprojT = np.ascontiguousarray(np.asarray(Wproj, np.float32).T).astype(
        ml_dtypes.bfloat16)                                            # (1024, 256)
    E = np.exp(np.asarray(ab, np.float32))[:, np.asarray(idxs)]    # (8, 320, 320)
    eb4 = np.zeros((2, 3, 128, 4 * N), dtype=ml_dtypes.bfloat16)
    for g in range(2):
        for mc in range(3):
            ms = MCS[mc]
            mb = 64 if mc == 2 else 0
            for hh in range(4):
                eb4[g, mc, mb:mb + ms, hh * N:(hh + 1) * N] = \
                    E[4 * g + hh, 128 * mc:128 * mc + ms, :].astype(
                        ml_dtypes.bfloat16)
    common = {
        "wqkvT": wqkvT, "wprojT": wprojT, "eb4": eb4,
        "g1c": g1c, "b1c": b1c,
        "g2c": np.ascontiguousarray(
            np.asarray(g2, np.float32).reshape(2, 128).T),
        "b2c": np.ascontiguousarray(
            np.asarray(b2, np.float32).reshape(2, 128).T),
    }
    in_maps = []
    for c in range(NCORES):
        m = dict(common)
        xs = x[c * BL:(c + 1) * BL].reshape(R, C)
        m["xT"] = np.ascontiguousarray(xs.T).astype(ml_dtypes.bfloat16)
        xb = np.ones((R, C + 1), dtype=ml_dtypes.bfloat16)
        xb[:, :C] = xs.astype(ml_dtypes.bfloat16)
        m["xb"] = xb
        in_maps.append(m)
    return in_maps


def _run(in_maps, trace=False):
    nc = _get_prog()
    res = run_bass_kernel_spmd(nc, in_maps, core_ids=list(range(NCORES)),
                               trace=trace)
    out = np.concatenate(
        [np.asarray(res.results[c]["yT"]).T.reshape(BL, N, C)
         for c in range(NCORES)], axis=0)
    return out.astype(np.float32), res


def kernel(**inputs):
    out, _ = _run(_host_prep(**inputs))
    return out


def run_traced(**inputs):
    return _run(_host_prep(**inputs), trace=True)


# revision 20
# speedup vs baseline: 1.0641x; 1.0641x over previous
"""Trainium2 Bass kernel for nn_Attention (qkv+BN -> biased softmax attention -> gelu -> proj+BN).

Sharding: data-parallel over batch B=128 across 8 NeuronCores (16 batches each).
BatchNorm (training-mode) statistics are all-reduced across cores (tiny collectives).

Per-core layout strategy:
  - x transposed on HOST to xT [C, rows] bf16 (no on-chip transposes).
  - qkv computed as [h_dim, row] with HOST-permuted Wqkv so the 1536 h-dims land
    as [q(8x32) | k(8x32) | v(8x128)] -> chunks 0-1 q, 2-3 k, 4-11 v (one v chunk per head).
  - v additionally produced ROW-major per batch (vb tiles [m, dv]) by small
    matmuls from xT, replacing the per-tile DMA transposes of v.
  - softmax(s+bias) = exp(s)*exp(bias) / rowsum: exp(bias) gathered on host (replicated),
    multiplied on DVE/GpSimd; rowsums via ones-matmul on PE; divide deferred past AV.
  - attention loop is batch-outer / head-inner; AV output overwrites that head's
    v slice in SBUF.
  - divide+gelu+proj+BN2 in a final phase (single activation-table switch to gelu).
"""
import os
import numpy as np
import ml_dtypes

import concourse.bass as bass
import concourse.tile as tile
from concourse import bacc, mybir
from concourse.bass_utils import run_bass_kernel_spmd

NCORES = int(os.environ.get("KERN_NCORES", "8"))
DBG_STOP = os.environ.get("KERN_STOP", "")   # "A" or "B" to stop early
TRACE_SIM = os.environ.get("KERN_TRACE_SIM", "") == "1"
B, N, C = 128, 320, 256
NH, DK, DV = 8, 32, 128
H = NH * (2 * DK + DV)       # 1536
DH = NH * DV                 # 1024
BL = B // 8                  # 16 batches/core (fixed shard size)
R = BL * N                   # 5120 rows/core
NT = B * N                   # 40960 global rows
EPS = 1e-5
SCALE = DK ** -0.5
FP = mybir.dt.float32
BF = mybir.dt.bfloat16

NHC = H // 128               # 12 h-chunks
NRB = R // 512               # 10 row blocks of 512
MCS = [128, 128, 64]         # chunking of N=320
AF = mybir.ActivationFunctionType
OP = mybir.AluOpType


def build_program():
    nc = bacc.Bacc("TRN2", target_bir_lowering=False, debug=False,
                   enable_asserts=False, num_devices=NCORES)
    xT_d = nc.dram_tensor("xT", [C, R], BF, kind="ExternalInput").ap()
    xb_d = nc.dram_tensor("xb", [R, C + 1], BF, kind="ExternalInput").ap()
    wqkvT_d = nc.dram_tensor("wqkvT", [C, H], BF, kind="ExternalInput").ap()
    wprojT_d = nc.dram_tensor("wprojT", [DH, C], BF, kind="ExternalInput").ap()
    eb4_d = nc.dram_tensor("eb4", [2, 3, 128, 4 * N], BF, kind="ExternalInput").ap()
    g1_d = nc.dram_tensor("g1c", [128, NHC], FP, kind="ExternalInput").ap()
    b1_d = nc.dram_tensor("b1c", [128, NHC], FP, kind="ExternalInput").ap()
    g2_d = nc.dram_tensor("g2c", [128, 2], FP, kind="ExternalInput").ap()
    b2_d = nc.dram_tensor("b2c", [128, 2], FP, kind="ExternalInput").ap()
    yT_d = nc.dram_tensor("yT", [C, R], FP, kind="ExternalOutput").ap()

    with tile.TileContext(nc, trace_sim=TRACE_SIM) as tc:
        with tc.tile_pool(name="const", bufs=1) as constp, \
             tc.tile_pool(name="qkv", bufs=1) as qkvp, \
             tc.tile_pool(name="dram", bufs=1, space="DRAM") as dramp, \
             tc.tile_pool(name="stat", bufs=1) as statp:

            # ---- constants ----
            wprojT_sb = constp.tile([128, NH * C], BF)     # 8 d-chunks side by side
            for dc in range(NH):
                nc.sync.dma_start(wprojT_sb[:, dc * C:(dc + 1) * C],
                                  wprojT_d[dc * 128:(dc + 1) * 128, :])
            g1_sb = constp.tile([128, NHC], FP)
            b1_sb = constp.tile([128, NHC], FP)
            g2_sb = constp.tile([128, 2], FP)
            b2_sb = constp.tile([128, 2], FP)
            nc.sync.dma_start(g1_sb[:], g1_d[:])
            nc.sync.dma_start(b1_sb[:], b1_d[:])
            nc.sync.dma_start(g2_sb[:], g2_d[:])
            nc.sync.dma_start(b2_sb[:], b2_d[:])
            ones_c = constp.tile([128, 1], BF)             # ones column (bf16 matmuls)
            nc.vector.memset(ones_c[:], 1.0)
            ones_rb = constp.tile([128, 128], BF)          # bf16 ones (outers)
            nc.vector.memset(ones_rb[:], 1.0)

            # ---- persistent big buffers ----
            qkv_sb = [qkvp.tile([128, R], BF, tag=f"qkv{i}", name=f"qkv{i}")
                      for i in range(NHC)]

            # xT / Wqkv columns live through Phase A + B (v-row matmuls), not C
            with tc.tile_pool(name="xa", bufs=1) as xp:
                xT_sb = [xp.tile([128, R], BF, tag=f"xT{cc}", name=f"xT{cc}")
                         for cc in range(2)]
                wq_sb = [xp.tile([128, H], BF, tag=f"wq{cc}", name=f"wq{cc}")
                         for cc in range(2)]
                for cc in range(2):
                    nc.sync.dma_start(xT_sb[cc][:], xT_d[cc * 128:(cc + 1) * 128, :])
                    nc.sync.dma_start(wq_sb[cc][:], wqkvT_d[cc * 128:(cc + 1) * 128, :])

                # ========== Phase A: q,k matmul + Gram-matrix BN1 stats ==========
                # sum_r qkv[h,r]  = Wqkv @ (sum_r x_r)        (xsum = G[:,256])
                # sum_r qkv[h,r]^2 = w_h^T (x^T x) w_h = sum_c WT[c,h]*(G@WT)[c,h]
                stats = statp.tile([128, 2 * NHC], FP)
                with tc.tile_pool(name="pa", bufs=2, space="PSUM") as pap:
                    # q,k chunks only (hc 0-3); v is produced row-major in Phase B
                    for rb in range(NRB):
                        for hc in range(4):
                            pq = pap.tile([128, 512], FP, tag="pq")
                            for cc in range(2):
                                nc.tensor.matmul(
                                    pq[:],
                                    wq_sb[cc][:, hc * 128:(hc + 1) * 128],
                                    xT_sb[cc][:, rb * 512:(rb + 1) * 512],
                                    start=(cc == 0), stop=(cc == 1))
                            nc.vector.tensor_copy(
                                qkv_sb[hc][:, rb * 512:(rb + 1) * 512], pq[:])
                    # Gram G = [x|1]^T [x|1], accumulated over 40 row chunks
                    Gsb = [statp.tile([128, C + 1], BF, tag=f"Gsb{_g}", name=f"Gsb{_g}")
                           for _g in range(2)]
                    with tc.tile_pool(name="gx", bufs=8) as gxp, \
                         tc.tile_pool(name="pg", bufs=1, space="PSUM") as pgp:
                        G_ps = [pgp.tile([128, C + 1], FP, tag=f"G{gg}", name=f"G{gg}")
                                for gg in range(2)]
                        for rc in range(40):
                            xbt = gxp.tile([128, C + 1], BF, tag="xb")
                            nc.sync.dma_start(xbt[:], xb_d[rc * 128:(rc + 1) * 128, :])
                            for gg in range(2):
                                nc.tensor.matmul(
                                    G_ps[gg][:], xbt[:, gg * 128:(gg + 1) * 128],
                                    xbt[:], start=(rc == 0), stop=(rc == 39))
                        for gg in range(2):
                            nc.vector.tensor_copy(Gsb[gg][:], G_ps[gg][:])
                    # P1 = G @ WT ; prod = P1 .* WT ; column-sum -> sumsq row.
                    # sums row from xsum (= Gsb[:,256]) @ WT. Rows land at
                    # partitions 0 (sums) / 32 (sumsq) of rows_ps[nc3].
                    rows_sb0 = statp.tile([1, H], BF, name="rows_sb0")  # per-h sums
                    rows_sb1 = statp.tile([1, H], BF, name="rows_sb1")  # per-h sumsq
                    with tc.tile_pool(name="pp1", bufs=2, space="PSUM") as pp1, \
                         tc.tile_pool(name="prw", bufs=3, space="PSUM") as prw, \
                         tc.tile_pool(name="pss", bufs=1, space="PSUM") as pssp, \
                         tc.tile_pool(name="sc", bufs=4) as scp:
                        rows_ps = [prw.tile([128, 512], FP, tag="rows", name=f"rows{_r}")
                                   for _r in range(3)]
                        for nc3 in range(3):
                            for gg in range(2):
                                p1 = pp1.tile([128, 512], FP, tag="p1")
                                for gp in range(2):
                                    nc.tensor.matmul(
                                        p1[:], Gsb[gp][:, gg * 128:(gg + 1) * 128],
                                        wq_sb[gp][:, nc3 * 512:(nc3 + 1) * 512],
                                        start=(gp == 0), stop=(gp == 1))
                                prod = scp.tile([128, 512], BF, tag="prod")
                                nc.vector.tensor_tensor(
                                    prod[:], p1[:],
                                    wq_sb[gg][:, nc3 * 512:(nc3 + 1) * 512], OP.mult)
                                nc.tensor.matmul(
                                    rows_ps[nc3][32:33, :], ones_c[:, 0:1], prod[:],
                                    start=(gg == 0), stop=(gg == 1),
                                    tile_position=(0, 32))
                            # separate accumulation group, AFTER sumsq completes
                            for gg in range(2):
                                nc.tensor.matmul(
                                    rows_ps[nc3][0:1, :], Gsb[gg][:, 256:257],
                                    wq_sb[gg][:, nc3 * 512:(nc3 + 1) * 512],
                                    start=(gg == 0), stop=(gg == 1),
                                    tile_position=(0, 0))
                            nc.vector.tensor_copy(
                                rows_sb0[0:1, nc3 * 512:(nc3 + 1) * 512],
                                rows_ps[nc3][0:1, :])
                            nc.vector.tensor_copy(
                                rows_sb1[0:1, nc3 * 512:(nc3 + 1) * 512],
                                rows_ps[nc3][32:33, :])
                        # transpose rows -> per-partition stat columns [128, 24]
                        stats_ps = pssp.tile([128, 2 * NHC], FP, tag="stp")
                        for hc in range(NHC):
                            nc.tensor.matmul(
                                stats_ps[:, hc:hc + 1],
                                rows_sb0[0:1, hc * 128:(hc + 1) * 128],
                                ones_c[0:1, 0:1])
                            nc.tensor.matmul(
                                stats_ps[:, NHC + hc:NHC + hc + 1],
                                rows_sb1[0:1, hc * 128:(hc + 1) * 128],
                                ones_c[0:1, 0:1])
                        nc.vector.tensor_copy(stats[:], stats_ps[:])
                bounce_i = dramp.tile([128, 2 * NHC], FP, tag="b1i")
                bounce_o = dramp.tile([128, 2 * NHC], FP, tag="b1o")
                nc.sync.dma_start(bounce_i[:], stats[:])
                nc.gpsimd.collective_compute(
                    "AllReduce", OP.add,
                    replica_groups=[list(range(NCORES))],
                    ins=[bounce_i.opt()], outs=[bounce_o.opt()])
                statsg = statp.tile([128, 2 * NHC], FP)
                nc.sync.dma_start(statsg[:], bounce_o[:])

                mean1 = statp.tile([128, NHC], FP)
                var1 = statp.tile([128, NHC], FP)
                tmp1 = statp.tile([128, NHC], FP)
                alpha1 = statp.tile([128, NHC], FP)
                beta1 = statp.tile([128, NHC], FP)
                nc.vector.tensor_scalar(mean1[:], statsg[:, 0:NHC], 1.0 / NT, None,
                                        OP.mult)
                nc.vector.tensor_scalar(var1[:], statsg[:, NHC:2 * NHC], 1.0 / NT,
                                        None, OP.mult)
                nc.vector.tensor_tensor(tmp1[:], mean1[:], mean1[:], OP.mult)
                nc.vector.tensor_tensor(var1[:], var1[:], tmp1[:], OP.subtract)
                nc.vector.tensor_scalar(var1[:], var1[:], EPS, None, OP.add)
                nc.scalar.activation(tmp1[:], var1[:], AF.Ln)
                nc.scalar.activation(var1[:], tmp1[:], AF.Exp, scale=-0.5)   # rstd
                nc.vector.tensor_tensor(alpha1[:], g1_sb[:], var1[:], OP.mult)
                nc.vector.tensor_tensor(beta1[:], mean1[:], alpha1[:], OP.mult)
                nc.vector.tensor_tensor(beta1[:], b1_sb[:], beta1[:], OP.subtract)
                for hc in range(4):                   # q,k only; v folded into gelu
                    nc.vector.tensor_scalar(
                        qkv_sb[hc][:], qkv_sb[hc][:],
                        alpha1[:, hc:hc + 1], beta1[:, hc:hc + 1], OP.mult, OP.add)

                if DBG_STOP not in ("A",):
                    # ========== Phase B: attention, batch-major ==========
                    # Per (g, mc): 4 heads' QK land concurrently (row groups
                    # 0/32/64/96) in one 4-bank PSUM tile; one batched exp
                    # (FD=1280) + one bias multiply cover all 4 heads.
                    with tc.tile_pool(name="ps4", bufs=1, space="PSUM") as ps4p, \
                         tc.tile_pool(name="pb", bufs=1, space="PSUM") as pbp, \
                         tc.tile_pool(name="prx", bufs=1, space="PSUM") as prxp, \
                         tc.tile_pool(name="prb", bufs=1, space="PSUM") as prbp, \
                         tc.tile_pool(name="pv", bufs=1, space="PSUM") as pvp, \
                         tc.tile_pool(name="eb2", bufs=4) as ep, \
                         tc.tile_pool(name="rrp", bufs=2) as rrp, \
                         tc.tile_pool(name="sc2", bufs=4) as scp2, \
                         tc.tile_pool(name="ebp", bufs=1) as ebp, \
                         tc.tile_pool(name="vb", bufs=6) as vbp:
                        eb4_sb = []                        # [g][mc] -> [128, 4*320]
                        for g in range(2):
                            row = []
                            for mc in range(3):
                                t = ebp.tile([128, 4 * N], BF, tag=f"eb{g}_{mc}",
                                             name=f"eb{g}_{mc}")
                                nc.sync.dma_start(t[:], eb4_d[g, mc])
                                row.append(t)
                            eb4_sb.append(row)
                        for b in range(BL):
                            # v row-major tiles for this batch: vb[g][mc] holds
                            # 4 heads' dv side by side at partitions [mb, mb+ms)
                            vb = [[None] * 3 for _ in range(2)]
                            for g in range(2):
                                for mc in range(3):
                                    ms = MCS[mc]
                                    mb = 64 if mc == 2 else 0
                                    vp_ps = pvp.tile([128, 512], FP, tag="vp")
                                    for cc in range(2):
                                        nc.tensor.matmul(
                                            vp_ps[mb:mb + ms, :],
                                            xT_sb[cc][:, b * N + 128 * mc:
                                                       b * N + 128 * mc + ms],
                                            wq_sb[cc][:, 512 + g * 512:
                                                      1024 + g * 512],
                                            start=(cc == 0), stop=(cc == 1),
                                            tile_position=(0, mb))
                                    vt = vbp.tile([128, 512], BF, tag="vb")
                                    nc.vector.tensor_copy(vt[mb:mb + ms, :],
                                                          vp_ps[mb:mb + ms, :])
                                    vb[g][mc] = vt
                            r_recip = []
                            for g in range(2):
                                et4s = []
                                for mc in range(3):
                                    ms = MCS[mc]
                                    mb = 64 if mc == 2 else 0
                                    s4 = ps4p.tile([128, 2048], FP, tag="s4")
                                    for hh in range(4):
                                        qr = 32 * hh
                                        nc.tensor.matmul(
                                            s4[mb:mb + ms, hh * 512:hh * 512 + N],
                                            qkv_sb[2 + g][qr:qr + 32,
                                                          b * N + 128 * mc:
                                                          b * N + 128 * mc + ms],
                                            qkv_sb[g][qr:qr + 32,
                                                      b * N:(b + 1) * N],
                                            tile_position=(qr, mb))
                                    et4 = ep.tile([128, 4 * N], BF, tag="et")
                                    sin = s4[mb:mb + ms, :].rearrange(
                                        "p (h n) -> p h n", h=4)[:, :, 0:N]
                                    eout = et4[mb:mb + ms, :].rearrange(
                                        "p (h n) -> p h n", h=4)
                                    nc.scalar.activation(eout, sin, AF.Exp,
                                                         scale=SCALE)
                                    nc.vector.tensor_tensor(
                                        et4[mb:mb + ms, :], et4[mb:mb + ms, :],
                                        eb4_sb[g][mc][mb:mb + ms, :], OP.mult)
                                    et4s.append(et4)
                                rp = prxp.tile([128, N], FP, tag="rx", name="rp")
                                for hh in range(4):
                                    rrow = 32 * hh
                                    for mc in range(3):
                                        ms = MCS[mc]
                                        mb = 64 if mc == 2 else 0
                                        nc.tensor.matmul(
                                            rp[rrow:rrow + 1, :],
                                            ones_c[mb:mb + ms, 0:1],
                                            et4s[mc][mb:mb + ms,
                                                     hh * N:(hh + 1) * N],
                                            start=(mc == 0), stop=(mc == 2),
                                            tile_position=(mb, rrow))
                                for hh in range(4):
                                    h = 4 * g + hh
                                    av = pbp.tile([128, N], FP, tag="av")
                                    for mc in range(3):
                                        ms = MCS[mc]
                                        mb = 64 if mc == 2 else 0
                                        nc.tensor.matmul(
                                            av[:],
                                            vb[g][mc][mb:mb + ms,
                                                      hh * 128:hh * 128 + 128],
                                            et4s[mc][mb:mb + ms,
                                                     hh * N:(hh + 1) * N],
                                            start=(mc == 0), stop=(mc == 2),
                                            tile_position=(mb, 0))
                                    if hh % 2:
                                        nc.scalar.copy(
                                            qkv_sb[4 + h][:, b * N:(b + 1) * N],
                                            av[:])
                                    else:
                                        nc.vector.tensor_copy(
                                            qkv_sb[4 + h][:, b * N:(b + 1) * N],
                                            av[:])
                                rr = rrp.tile([128, N], FP, tag="rr", name="rr")
                                nc.vector.reciprocal_approx_fast(rr[:], rp[:])
                                r_recip.append(rr)
                            # softmax divide for this batch
                            rrb16 = []
                            for g in range(2):
                                rb16 = scp2.tile([128, N], BF, tag="rb16",
                                                 name="rb16")
                                nc.vector.tensor_copy(rb16[:], r_recip[g][:])
                                rrb16.append(rb16)
                            for h in range(NH):
                                rr = rrb16[h // 4]
                                rbr = 32 * (h % 4)
                                rb_ps = prbp.tile([128, N], FP, tag="rbc",
                                                  name="rbps")
                                nc.tensor.matmul(
                                    rb_ps[:], ones_rb[rbr:rbr + 1, :],
                                    rr[rbr:rbr + 1, :], tile_position=(rbr, 0))
                                nc.vector.tensor_tensor(
                                    qkv_sb[4 + h][:, b * N:(b + 1) * N],
                                    qkv_sb[4 + h][:, b * N:(b + 1) * N],
                                    rb_ps[:], OP.mult)

            if DBG_STOP not in ("A", "B"):
                # ========== Phase C: gelu, transposed proj (yT), BN2 ==========
                # yT[c, r] = sum_dh WprojT[dh, c] * gelu(av)[dh, r]; BN2 stats
                # and affine become per-partition (c on partitions).
                with tc.tile_pool(name="ppy", bufs=4, space="PSUM") as ppy, \
                     tc.tile_pool(name="yb", bufs=1) as yp, \
                     tc.tile_pool(name="sc3", bufs=4) as scp3:
                    for h in range(NH):
                        nc.scalar.activation(qkv_sb[4 + h][:], qkv_sb[4 + h][:],
                                             AF.Gelu,
                                             scale=alpha1[:, 4 + h:5 + h],
                                             bias=beta1[:, 4 + h:5 + h])
                    yT_sb = [yp.tile([128, R], BF, tag=f"yT{c2}", name=f"yT{c2}")
                             for c2 in range(2)]
                    sq_sb = yp.tile([128, R], BF, name="sq_sb")   # square scratch
                    s2sum = statp.tile([128, 2 * NRB], FP)
                    s2sq = statp.tile([128, 2], FP)
                    for rb in range(NRB):
                        for c2 in range(2):
                            py = ppy.tile([128, 512], FP, tag="py")
                            for h in range(NH):
                                nc.tensor.matmul(
                                    py[:],
                                    wprojT_sb[:, h * C + c2 * 128:
                                              h * C + c2 * 128 + 128],
                                    qkv_sb[4 + h][:, rb * 512:(rb + 1) * 512],
                                    start=(h == 0), stop=(h == NH - 1))
                            nc.vector.tensor_scalar(
                                yT_sb[c2][:, rb * 512:(rb + 1) * 512], py[:],
                                1.0, 0.0, OP.mult, OP.add,
                                accum_out=s2sum[:, c2 * NRB + rb:
                                                c2 * NRB + rb + 1])
                    for c2 in range(2):
                        nc.scalar.activation(sq_sb[:], yT_sb[c2][:], AF.Square,
                                             accum_out=s2sq[:, c2:c2 + 1])
                    st2 = statp.tile([128, 4], FP)
                    for c2 in range(2):
                        nc.vector.tensor_reduce(
                            st2[:, c2:c2 + 1], s2sum[:, c2 * NRB:(c2 + 1) * NRB],
                            mybir.AxisListType.X, OP.add)
                    nc.vector.tensor_copy(st2[:, 2:4], s2sq[:])
                    b2i = dramp.tile([128, 4], FP, tag="b2i")
                    b2o = dramp.tile([128, 4], FP, tag="b2o")
                    nc.sync.dma_start(b2i[:], st2[:])
                    nc.gpsimd.collective_compute(
                        "AllReduce", OP.add,
                        replica_groups=[list(range(NCORES))],
                        ins=[b2i.opt()], outs=[b2o.opt()])
                    st2g = statp.tile([128, 4], FP)
                    nc.sync.dma_start(st2g[:], b2o[:])

                    mean2 = statp.tile([128, 2], FP)
                    var2 = statp.tile([128, 2], FP)
                    tmp2 = statp.tile([128, 2], FP)
                    alpha2 = statp.tile([128, 2], FP)
                    beta2 = statp.tile([128, 2], FP)
                    nc.vector.tensor_scalar(mean2[:], st2g[:, 0:2], 1.0 / NT, None,
                                            OP.mult)
                    nc.vector.tensor_scalar(var2[:], st2g[:, 2:4], 1.0 / NT,
                                            None, OP.mult)
                    nc.vector.tensor_tensor(tmp2[:], mean2[:], mean2[:], OP.mult)
                    nc.vector.tensor_tensor(var2[:], var2[:], tmp2[:], OP.subtract)
                    nc.vector.tensor_scalar(var2[:], var2[:], EPS, None, OP.add)
                    nc.scalar.activation(tmp2[:], var2[:], AF.Ln)
                    nc.scalar.activation(var2[:], tmp2[:], AF.Exp, scale=-0.5)
                    nc.vector.tensor_tensor(alpha2[:], g2_sb[:], var2[:], OP.mult)
                    nc.vector.tensor_tensor(beta2[:], mean2[:], alpha2[:], OP.mult)
                    nc.vector.tensor_tensor(beta2[:], b2_sb[:], beta2[:],
                                            OP.subtract)

                    for rb in range(NRB):
                        for c2 in range(2):
                            yo = scp3.tile([128, 512], FP, tag="yo")
                            nc.vector.tensor_scalar(
                                yo[:], yT_sb[c2][:, rb * 512:(rb + 1) * 512],
                                alpha2[:, c2:c2 + 1], beta2[:, c2:c2 + 1],
                                OP.mult, OP.add)
                            nc.sync.dma_start(
                                yT_d[c2 * 128:(c2 + 1) * 128,
                                     rb * 512:(rb + 1) * 512], yo[:])
            if DBG_STOP in ("A", "B"):
                dsrc = qkv_sb[0] if DBG_STOP == "A" else qkv_sb[4]
                for i in range(20):
                    dq = statp.tile([128, C], FP, tag="dq", name="dq", bufs=2)
                    nc.vector.tensor_copy(dq[:], dsrc[:, i * C:(i + 1) * C])
                    nc.sync.dma_start(yT_d[0:128, i * C:(i + 1) * C], dq[:])

    nc.compile()
    return nc


_PROG = None


def _get_prog():
    global _PROG
    if _PROG is None:
        _PROG = build_program()
    return _PROG


def _host_prep(x, Wqkv, g1, b1, ab, Wproj, g2, b2, idxs):
    perm = np.empty(H, dtype=np.int64)
    for h in range(NH):
        base = h * (2 * DK + DV)
        perm[DK * h: DK * (h + 1)] = np.arange(base, base + DK)
        perm[NH * DK + DK * h: NH * DK + DK * (h + 1)] = \
            np.arange(base + DK, base + 2 * DK)
        perm[2 * NH * DK + DV * h: 2 * NH * DK + DV * (h + 1)] = \
            np.arange(base + 2 * DK, base + 2 * DK + DV)
    x = np.asarray(x, dtype=np.float32)
    Wqkv = np.asarray(Wqkv, dtype=np.float32)
    wqkvT = np.ascontiguousarray(Wqkv[perm, :].T).astype(ml_dtypes.bfloat16)
    g1c = np.ascontiguousarray(np.asarray(g1, np.float32)[perm].reshape(NHC, 128).T)
    b1c = np.ascontiguousarray(np.asarray(b1, np.float32)[perm].reshape(NHC, 128).T)
    wprojT = np.ascontiguousarray(np.asarray(Wproj, np.float32).T).astype(
        ml_dtypes.bfloat16)                                            # (1024, 256)
    E = np.exp(np.asarray(ab, np.float32))[:, np.asarray(idxs)]    # (8, 320, 320)
    eb4 = np.zeros((2, 3, 128, 4 * N), dtype=ml_dtypes.bfloat16)
    for g in range(2):
        for mc in range(3):
            ms = MCS[mc]
            mb = 64 if mc == 2 else 0
            for hh in range(4):
                eb4[g, mc, mb:mb + ms, hh * N:(hh + 1) * N] = \
                    E[4 * g + hh, 128 * mc:128 * mc + ms, :].astype(
                        ml_dtypes.bfloat16)
    common = {
        "wqkvT": wqkvT, "wprojT": wprojT, "eb4": eb4,
        "g1c": g1c, "b1c": b1c,
        "g2c": np.ascontiguousarray(
            np.asarray(g2, np.float32).reshape(2, 128).T),
        "b2c": np.ascontiguousarray(
            np.asarray(b2, np.float32).reshape(2, 128).T),
    }
    in_maps = []
    for c in range(NCORES):
        m = dict(common)
        xs = x[c * BL:(c + 1) * BL].reshape(R, C)
        m["xT"] = np.ascontiguousarray(xs.T).astype(ml_dtypes.bfloat16)
        xb = np.ones((R, C + 1), dtype=ml_dtypes.bfloat16)
        xb[:, :C] = xs.astype(ml_dtypes.bfloat16)
        m["xb"] = xb
        in_maps.append(m)
    return in_maps


def _run(in_maps, trace=False):
    nc = _get_prog()
    res = run_bass_kernel_spmd(nc, in_maps, core_ids=list(range(NCORES)),
                               trace=trace)
    out = np.concatenate(
        [np.asarray(res.results[c]["yT"]).T.reshape(BL, N, C)
         for c in range(NCORES)], axis=0)
    return out.astype(np.float32), res


def kernel(**inputs):
    out, _ = _run(_host_prep(**inputs))
    return out


def run_traced(**inputs):
    return _run(_host_prep(**inputs), trace=True)


# revision 22
# speedup vs baseline: 1.1138x; 1.0468x over previous
"""Trainium2 Bass kernel for nn_Attention (qkv+BN -> biased softmax attention -> gelu -> proj+BN).

Sharding: data-parallel over batch B=128 across 8 NeuronCores (16 batches each).
BatchNorm (training-mode) statistics are all-reduced across cores (tiny collectives).

Per-core layout strategy:
  - x transposed on HOST to xT [C, rows] bf16 (no on-chip transposes).
  - qkv computed as [h_dim, row] with HOST-permuted Wqkv so the 1536 h-dims land
    as [q(8x32) | k(8x32) | v(8x128)] -> chunks 0-1 q, 2-3 k, 4-11 v (one v chunk per head).
  - v additionally produced ROW-major per batch (vb tiles [m, dv]) by small
    matmuls from xT, replacing the per-tile DMA transposes of v.
  - softmax(s+bias) = exp(s)*exp(bias) / rowsum: exp(bias) gathered on host (replicated),
    multiplied on DVE/GpSimd; rowsums via ones-matmul on PE; divide deferred past AV.
  - attention loop is batch-outer / head-inner; AV output overwrites that head's
    v slice in SBUF.
  - divide+gelu+proj+BN2 in a final phase (single activation-table switch to gelu).
"""
import os
import numpy as np
import ml_dtypes

import concourse.bass as bass
import concourse.tile as tile
from concourse import bacc, mybir
from concourse.bass_utils import run_bass_kernel_spmd

NCORES = int(os.environ.get("KERN_NCORES", "8"))
DBG_STOP = os.environ.get("KERN_STOP", "")   # "A" or "B" to stop early
TRACE_SIM = os.environ.get("KERN_TRACE_SIM", "") == "1"
B, N, C = 128, 320, 256
NH, DK, DV = 8, 32, 128
H = NH * (2 * DK + DV)       # 1536
DH = NH * DV                 # 1024
BL = B // 8                  # 16 batches/core (fixed shard size)
R = BL * N                   # 5120 rows/core
NT = B * N                   # 40960 global rows
EPS = 1e-5
SCALE = DK ** -0.5
FP = mybir.dt.float32
BF = mybir.dt.bfloat16

NHC = H // 128               # 12 h-chunks
NRB = R // 512               # 10 row blocks of 512
MCS = [128, 128, 64]         # chunking of N=320
AF = mybir.ActivationFunctionType
OP = mybir.AluOpType


def build_program():
    nc = bacc.Bacc("TRN2", target_bir_lowering=False, debug=False,
                   enable_asserts=False, num_devices=NCORES)
    xT_d = nc.dram_tensor("xT", [C, R], BF, kind="ExternalInput").ap()
    xb_d = nc.dram_tensor("xb", [R, C + 1], BF, kind="ExternalInput").ap()
    wqkvT_d = nc.dram_tensor("wqkvT", [C, H], BF, kind="ExternalInput").ap()
    wprojT_d = nc.dram_tensor("wprojT", [DH, C], BF, kind="ExternalInput").ap()
    eb4_d = nc.dram_tensor("eb4", [2, 3, 128, 4 * N], BF, kind="ExternalInput").ap()
    g1_d = nc.dram_tensor("g1c", [128, NHC], FP, kind="ExternalInput").ap()
    b1_d = nc.dram_tensor("b1c", [128, NHC], FP, kind="ExternalInput").ap()
    g2_d = nc.dram_tensor("g2c", [128, 2], FP, kind="ExternalInput").ap()
    b2_d = nc.dram_tensor("b2c", [128, 2], FP, kind="ExternalInput").ap()
    yT_d = nc.dram_tensor("yT", [C, R], FP, kind="ExternalOutput").ap()

    with tile.TileContext(nc, trace_sim=TRACE_SIM) as tc:
        with tc.tile_pool(name="const", bufs=1) as constp, \
             tc.tile_pool(name="qkv", bufs=1) as qkvp, \
             tc.tile_pool(name="dram", bufs=1, space="DRAM") as dramp, \
             tc.tile_pool(name="stat", bufs=1) as statp:

            # ---- constants ----
            wprojT_sb = constp.tile([128, NH * C], BF)     # 8 d-chunks side by side
            for dc in range(NH):
                nc.sync.dma_start(wprojT_sb[:, dc * C:(dc + 1) * C],
                                  wprojT_d[dc * 128:(dc + 1) * 128, :])
            g1_sb = constp.tile([128, NHC], FP)
            b1_sb = constp.tile([128, NHC], FP)
            g2_sb = constp.tile([128, 2], FP)
            b2_sb = constp.tile([128, 2], FP)
            nc.sync.dma_start(g1_sb[:], g1_d[:])
            nc.sync.dma_start(b1_sb[:], b1_d[:])
            nc.sync.dma_start(g2_sb[:], g2_d[:])
            nc.sync.dma_start(b2_sb[:], b2_d[:])
            ones_c = constp.tile([128, 1], BF)             # ones column (bf16 matmuls)
            nc.vector.memset(ones_c[:], 1.0)
            ones_rb = constp.tile([128, 128], BF)          # bf16 ones (outers)
            nc.vector.memset(ones_rb[:], 1.0)

            # ---- persistent big buffers ----
            qkv_sb = [qkvp.tile([128, R], BF, tag=f"qkv{i}", name=f"qkv{i}")
                      for i in range(NHC)]

            # xT / Wqkv columns live through Phase A + B (v-row matmuls), not C
            with tc.tile_pool(name="xa", bufs=1) as xp:
                xT_sb = [xp.tile([128, R], BF, tag=f"xT{cc}", name=f"xT{cc}")
                         for cc in range(2)]
                wq_sb = [xp.tile([128, H], BF, tag=f"wq{cc}", name=f"wq{cc}")
                         for cc in range(2)]
                for cc in range(2):
                    nc.sync.dma_start(xT_sb[cc][:], xT_d[cc * 128:(cc + 1) * 128, :])
                    nc.sync.dma_start(wq_sb[cc][:], wqkvT_d[cc * 128:(cc + 1) * 128, :])

                # ========== Phase A: q,k matmul + Gram-matrix BN1 stats ==========
                # sum_r qkv[h,r]  = Wqkv @ (sum_r x_r)        (xsum = G[:,256])
                # sum_r qkv[h,r]^2 = w_h^T (x^T x) w_h = sum_c WT[c,h]*(G@WT)[c,h]
                stats = statp.tile([128, 2 * NHC], FP)
                with tc.tile_pool(name="pa", bufs=2, space="PSUM") as pap:
                    # q,k chunks only (hc 0-3); v is produced row-major in Phase B
                    for rb in range(NRB):
                        for hc in range(4):
                            pq = pap.tile([128, 512], FP, tag="pq")
                            for cc in range(2):
                                nc.tensor.matmul(
                                    pq[:],
                                    wq_sb[cc][:, hc * 128:(hc + 1) * 128],
                                    xT_sb[cc][:, rb * 512:(rb + 1) * 512],
                                    start=(cc == 0), stop=(cc == 1))
                            if hc % 2:
                                nc.scalar.copy(
                                    qkv_sb[hc][:, rb * 512:(rb + 1) * 512], pq[:])
                            else:
                                nc.vector.tensor_copy(
                                    qkv_sb[hc][:, rb * 512:(rb + 1) * 512], pq[:])
                    # Gram G = [x|1]^T [x|1], accumulated over 40 row chunks
                    Gsb = [statp.tile([128, C + 1], BF, tag=f"Gsb{_g}", name=f"Gsb{_g}")
                           for _g in range(2)]
                    with tc.tile_pool(name="gx", bufs=8) as gxp, \
                         tc.tile_pool(name="pg", bufs=1, space="PSUM") as pgp:
                        G_ps = [pgp.tile([128, C + 1], FP, tag=f"G{gg}", name=f"G{gg}")
                                for gg in range(2)]
                        for rc in range(40):
                            xbt = gxp.tile([128, C + 1], BF, tag="xb")
                            nc.sync.dma_start(xbt[:], xb_d[rc * 128:(rc + 1) * 128, :])
                            for gg in range(2):
                                nc.tensor.matmul(
                                    G_ps[gg][:], xbt[:, gg * 128:(gg + 1) * 128],
                                    xbt[:], start=(rc == 0), stop=(rc == 39))
                        for gg in range(2):
                            nc.vector.tensor_copy(Gsb[gg][:], G_ps[gg][:])
                    # P1 = G @ WT ; prod = P1 .* WT ; column-sum -> sumsq row.
                    # sums row from xsum (= Gsb[:,256]) @ WT. Rows land at
                    # partitions 0 (sums) / 32 (sumsq) of rows_ps[nc3].
                    rows_sb0 = statp.tile([1, H], BF, name="rows_sb0")  # per-h sums
                    rows_sb1 = statp.tile([1, H], BF, name="rows_sb1")  # per-h sumsq
                    with tc.tile_pool(name="pp1", bufs=2, space="PSUM") as pp1, \
                         tc.tile_pool(name="prw", bufs=3, space="PSUM") as prw, \
                         tc.tile_pool(name="pss", bufs=1, space="PSUM") as pssp, \
                         tc.tile_pool(name="sc", bufs=4) as scp:
                        rows_ps = [prw.tile([128, 512], FP, tag="rows", name=f"rows{_r}")
                                   for _r in range(3)]
                        for nc3 in range(3):
                            for gg in range(2):
                                p1 = pp1.tile([128, 512], FP, tag="p1")
                                for gp in range(2):
                                    nc.tensor.matmul(
                                        p1[:], Gsb[gp][:, gg * 128:(gg + 1) * 128],
                                        wq_sb[gp][:, nc3 * 512:(nc3 + 1) * 512],
                                        start=(gp == 0), stop=(gp == 1))
                                prod = scp.tile([128, 512], BF, tag="prod")
                                nc.vector.tensor_tensor(
                                    prod[:], p1[:],
                                    wq_sb[gg][:, nc3 * 512:(nc3 + 1) * 512], OP.mult)
                                nc.tensor.matmul(
                                    rows_ps[nc3][32:33, :], ones_c[:, 0:1], prod[:],
                                    start=(gg == 0), stop=(gg == 1),
                                    tile_position=(0, 32))
                            # separate accumulation group, AFTER sumsq completes
                            for gg in range(2):
                                nc.tensor.matmul(
                                    rows_ps[nc3][0:1, :], Gsb[gg][:, 256:257],
                                    wq_sb[gg][:, nc3 * 512:(nc3 + 1) * 512],
                                    start=(gg == 0), stop=(gg == 1),
                                    tile_position=(0, 0))
                            nc.vector.tensor_copy(
                                rows_sb0[0:1, nc3 * 512:(nc3 + 1) * 512],
                                rows_ps[nc3][0:1, :])
                            nc.vector.tensor_copy(
                                rows_sb1[0:1, nc3 * 512:(nc3 + 1) * 512],
                                rows_ps[nc3][32:33, :])
                        # transpose rows -> per-partition stat columns [128, 24]
                        stats_ps = pssp.tile([128, 2 * NHC], FP, tag="stp")
                        for hc in range(NHC):
                            nc.tensor.matmul(
                                stats_ps[:, hc:hc + 1],
                                rows_sb0[0:1, hc * 128:(hc + 1) * 128],
                                ones_c[0:1, 0:1])
                            nc.tensor.matmul(
                                stats_ps[:, NHC + hc:NHC + hc + 1],
                                rows_sb1[0:1, hc * 128:(hc + 1) * 128],
                                ones_c[0:1, 0:1])
                        nc.vector.tensor_copy(stats[:], stats_ps[:])
                bounce_i = dramp.tile([128, 2 * NHC], FP, tag="b1i")
                bounce_o = dramp.tile([128, 2 * NHC], FP, tag="b1o")
                nc.sync.dma_start(bounce_i[:], stats[:])
                nc.gpsimd.collective_compute(
                    "AllReduce", OP.add,
                    replica_groups=[list(range(NCORES))],
                    ins=[bounce_i.opt()], outs=[bounce_o.opt()])
                statsg = statp.tile([128, 2 * NHC], FP)
                nc.sync.dma_start(statsg[:], bounce_o[:])

                mean1 = statp.tile([128, NHC], FP)
                var1 = statp.tile([128, NHC], FP)
                tmp1 = statp.tile([128, NHC], FP)
                alpha1 = statp.tile([128, NHC], FP)
                beta1 = statp.tile([128, NHC], FP)
                nc.vector.tensor_scalar(mean1[:], statsg[:, 0:NHC], 1.0 / NT, None,
                                        OP.mult)
                nc.vector.tensor_scalar(var1[:], statsg[:, NHC:2 * NHC], 1.0 / NT,
                                        None, OP.mult)
                nc.vector.tensor_tensor(tmp1[:], mean1[:], mean1[:], OP.mult)
                nc.vector.tensor_tensor(var1[:], var1[:], tmp1[:], OP.subtract)
                nc.vector.tensor_scalar(var1[:], var1[:], EPS, None, OP.add)
                nc.scalar.activation(tmp1[:], var1[:], AF.Ln)
                nc.scalar.activation(var1[:], tmp1[:], AF.Exp, scale=-0.5)   # rstd
                nc.vector.tensor_tensor(alpha1[:], g1_sb[:], var1[:], OP.mult)
                nc.vector.tensor_tensor(beta1[:], mean1[:], alpha1[:], OP.mult)
                nc.vector.tensor_tensor(beta1[:], b1_sb[:], beta1[:], OP.subtract)
                for hc in range(4):                   # q,k only; v folded into gelu
                    nc.vector.tensor_scalar(
                        qkv_sb[hc][:], qkv_sb[hc][:],
                        alpha1[:, hc:hc + 1], beta1[:, hc:hc + 1], OP.mult, OP.add)

                if DBG_STOP not in ("A",):
                    # ========== Phase B: attention, batch-major ==========
                    # Per (g, mc): 4 heads' QK land concurrently (row groups
                    # 0/32/64/96) in one 4-bank PSUM tile; one batched exp
                    # (FD=1280) + one bias multiply cover all 4 heads.
                    with tc.tile_pool(name="ps4", bufs=1, space="PSUM") as ps4p, \
                         tc.tile_pool(name="pb", bufs=1, space="PSUM") as pbp, \
                         tc.tile_pool(name="prx", bufs=1, space="PSUM") as prxp, \
                         tc.tile_pool(name="prb", bufs=1, space="PSUM") as prbp, \
                         tc.tile_pool(name="pv", bufs=1, space="PSUM") as pvp, \
                         tc.tile_pool(name="eb2", bufs=4) as ep, \
                         tc.tile_pool(name="rrp", bufs=2) as rrp, \
                         tc.tile_pool(name="sc2", bufs=4) as scp2, \
                         tc.tile_pool(name="ebp", bufs=1) as ebp, \
                         tc.tile_pool(name="vb", bufs=6) as vbp:
                        eb4_sb = []                        # [g][mc] -> [128, 4*320]
                        for g in range(2):
                            row = []
                            for mc in range(3):
                                t = ebp.tile([128, 4 * N], BF, tag=f"eb{g}_{mc}",
                                             name=f"eb{g}_{mc}")
                                nc.sync.dma_start(t[:], eb4_d[g, mc])
                                row.append(t)
                            eb4_sb.append(row)
                        for b in range(BL):
                            # v row-major tiles for this batch: vb[g][mc] holds
                            # 4 heads' dv side by side at partitions [mb, mb+ms)
                            vb = [[None] * 3 for _ in range(2)]
                            for g in range(2):
                                for mc in range(3):
                                    ms = MCS[mc]
                                    mb = 64 if mc == 2 else 0
                                    vp_ps = pvp.tile([128, 512], FP, tag="vp")
                                    for cc in range(2):
                                        nc.tensor.matmul(
                                            vp_ps[mb:mb + ms, :],
                                            xT_sb[cc][:, b * N + 128 * mc:
                                                       b * N + 128 * mc + ms],
                                            wq_sb[cc][:, 512 + g * 512:
                                                      1024 + g * 512],
                                            start=(cc == 0), stop=(cc == 1),
                                            tile_position=(0, mb))
                                    vt = vbp.tile([128, 512], BF, tag="vb")
                                    if g:
                                        nc.scalar.copy(vt[mb:mb + ms, :],
                                                       vp_ps[mb:mb + ms, :])
                                    else:
                                        nc.vector.tensor_copy(
                                            vt[mb:mb + ms, :],
                                            vp_ps[mb:mb + ms, :])
                                    vb[g][mc] = vt
                            r_recip = []
                            for g in range(2):
                                et4s = []
                                for mc in range(3):
                                    ms = MCS[mc]
                                    mb = 64 if mc == 2 else 0
                                    s4 = ps4p.tile([128, 2048], FP, tag="s4")
                                    for hh in range(4):
                                        qr = 32 * hh
                                        nc.tensor.matmul(
                                            s4[mb:mb + ms, hh * 512:hh * 512 + N],
                                            qkv_sb[2 + g][qr:qr + 32,
                                                          b * N + 128 * mc:
                                                          b * N + 128 * mc + ms],
                                            qkv_sb[g][qr:qr + 32,
                                                      b * N:(b + 1) * N],
                                            tile_position=(qr, mb))
                                    et4 = ep.tile([128, 4 * N], BF, tag="et")
                                    sin = s4[mb:mb + ms, :].rearrange(
                                        "p (h n) -> p h n", h=4)[:, :, 0:N]
                                    eout = et4[mb:mb + ms, :].rearrange(
                                        "p (h n) -> p h n", h=4)
                                    nc.scalar.activation(eout, sin, AF.Exp,
                                                         scale=SCALE)
                                    nc.vector.tensor_tensor(
                                        et4[mb:mb + ms, :], et4[mb:mb + ms, :],
                                        eb4_sb[g][mc][mb:mb + ms, :], OP.mult)
                                    et4s.append(et4)
                                rp = prxp.tile([128, N], FP, tag="rx", name="rp")
                                for hh in range(4):
                                    rrow = 32 * hh
                                    for mc in range(3):
                                        ms = MCS[mc]
                                        mb = 64 if mc == 2 else 0
                                        nc.tensor.matmul(
                                            rp[rrow:rrow + 1, :],
                                            ones_c[mb:mb + ms, 0:1],
                                            et4s[mc][mb:mb + ms,
                                                     hh * N:(hh + 1) * N],
                                            start=(mc == 0), stop=(mc == 2),
                                            tile_position=(mb, rrow))
                                for hh in range(4):
                                    h = 4 * g + hh
                                    av = pbp.tile([128, N], FP, tag="av")
                                    for mc in range(3):
                                        ms = MCS[mc]
                                        mb = 64 if mc == 2 else 0
                                        nc.tensor.matmul(
                                            av[:],
                                            vb[g][mc][mb:mb + ms,
                                                      hh * 128:hh * 128 + 128],
                                            et4s[mc][mb:mb + ms,
                                                     hh * N:(hh + 1) * N],
                                            start=(mc == 0), stop=(mc == 2),
                                            tile_position=(mb, 0))
                                    if hh % 2:
                                        nc.scalar.copy(
                                            qkv_sb[4 + h][:, b * N:(b + 1) * N],
                                            av[:])
                                    else:
                                        nc.vector.tensor_copy(
                                            qkv_sb[4 + h][:, b * N:(b + 1) * N],
                                            av[:])
                                rr = rrp.tile([128, N], FP, tag="rr", name="rr")
                                nc.vector.reciprocal_approx_fast(rr[:], rp[:])
                                r_recip.append(rr)
                            # softmax divide for this batch
                            rrb16 = []
                            for g in range(2):
                                rb16 = scp2.tile([128, N], BF, tag="rb16",
                                                 name="rb16")
                                nc.vector.tensor_copy(rb16[:], r_recip[g][:])
                                rrb16.append(rb16)
                            for h in range(NH):
                                rr = rrb16[h // 4]
                                rbr = 32 * (h % 4)
                                rb_ps = prbp.tile([128, N], FP, tag="rbc",
                                                  name="rbps")
                                nc.tensor.matmul(
                                    rb_ps[:], ones_rb[rbr:rbr + 1, :],
                                    rr[rbr:rbr + 1, :], tile_position=(rbr, 0))
                                nc.vector.tensor_tensor(
                                    qkv_sb[4 + h][:, b * N:(b + 1) * N],
                                    qkv_sb[4 + h][:, b * N:(b + 1) * N],
                                    rb_ps[:], OP.mult)

            if DBG_STOP not in ("A", "B"):
                # ========== Phase C: gelu, transposed proj (yT), BN2 ==========
                # yT[c, r] = sum_dh WprojT[dh, c] * gelu(av)[dh, r]; BN2 stats
                # and affine become per-partition (c on partitions).
                with tc.tile_pool(name="ppy", bufs=4, space="PSUM") as ppy, \
                     tc.tile_pool(name="yb", bufs=1) as yp, \
                     tc.tile_pool(name="sc3", bufs=4) as scp3:
                    for h in range(NH):
                        nc.scalar.activation(qkv_sb[4 + h][:], qkv_sb[4 + h][:],
                                             AF.Gelu,
                                             scale=alpha1[:, 4 + h:5 + h],
                                             bias=beta1[:, 4 + h:5 + h])
                    yT_sb = [yp.tile([128, R], BF, tag=f"yT{c2}", name=f"yT{c2}")
                             for c2 in range(2)]
                    sq_sb = yp.tile([128, R], BF, name="sq_sb")   # square scratch
                    s2sum = statp.tile([128, 2 * NRB], FP)
                    s2sq = statp.tile([128, 2], FP)
                    for rb in range(NRB):
                        for c2 in range(2):
                            py = ppy.tile([128, 512], FP, tag="py")
                            for h in range(NH):
                                nc.tensor.matmul(
                                    py[:],
                                    wprojT_sb[:, h * C + c2 * 128:
                                              h * C + c2 * 128 + 128],
                                    qkv_sb[4 + h][:, rb * 512:(rb + 1) * 512],
                                    start=(h == 0), stop=(h == NH - 1))
                            nc.vector.tensor_scalar(
                                yT_sb[c2][:, rb * 512:(rb + 1) * 512], py[:],
                                1.0, 0.0, OP.mult, OP.add,
                                accum_out=s2sum[:, c2 * NRB + rb:
                                                c2 * NRB + rb + 1])
                    for c2 in range(2):
                        nc.scalar.activation(sq_sb[:], yT_sb[c2][:], AF.Square,
                                             accum_out=s2sq[:, c2:c2 + 1])
                    st2 = statp.tile([128, 4], FP)
                    for c2 in range(2):
                        nc.vector.tensor_reduce(
                            st2[:, c2:c2 + 1], s2sum[:, c2 * NRB:(c2 + 1) * NRB],
                            mybir.AxisListType.X, OP.add)
                    nc.vector.tensor_copy(st2[:, 2:4], s2sq[:])
                    b2i = dramp.tile([128, 4], FP, tag="b2i")
                    b2o = dramp.tile([128, 4], FP, tag="b2o")
                    nc.sync.dma_start(b2i[:], st2[:])
                    nc.gpsimd.collective_compute(
                        "AllReduce", OP.add,
                        replica_groups=[list(range(NCORES))],
                        ins=[b2i.opt()], outs=[b2o.opt()])
                    st2g = statp.tile([128, 4], FP)
                    nc.sync.dma_start(st2g[:], b2o[:])

                    mean2 = statp.tile([128, 2], FP)
                    var2 = statp.tile([128, 2], FP)
                    tmp2 = statp.tile([128, 2], FP)
                    alpha2 = statp.tile([128, 2], FP)
                    beta2 = statp.tile([128, 2], FP)
                    nc.vector.tensor_scalar(mean2[:], st2g[:, 0:2], 1.0 / NT, None,
                                            OP.mult)
                    nc.vector.tensor_scalar(var2[:], st2g[:, 2:4], 1.0 / NT,
                                            None, OP.mult)
                    nc.vector.tensor_tensor(tmp2[:], mean2[:], mean2[:], OP.mult)
                    nc.vector.tensor_tensor(var2[:], var2[:], tmp2[:], OP.subtract)
                    nc.vector.tensor_scalar(var2[:], var2[:], EPS, None, OP.add)
                    nc.scalar.activation(tmp2[:], var2[:], AF.Ln)
                    nc.scalar.activation(var2[:], tmp2[:], AF.Exp, scale=-0.5)
                    nc.vector.tensor_tensor(alpha2[:], g2_sb[:], var2[:], OP.mult)
                    nc.vector.tensor_tensor(beta2[:], mean2[:], alpha2[:], OP.mult)
                    nc.vector.tensor_tensor(beta2[:], b2_sb[:], beta2[:],
                                            OP.subtract)

                    for rb in range(NRB):
                        for c2 in range(2):
                            yo = scp3.tile([128, 512], FP, tag="yo")
                            nc.vector.tensor_scalar(
                                yo[:], yT_sb[c2][:, rb * 512:(rb + 1) * 512],
                                alpha2[:, c2:c2 + 1], beta2[:, c2:c2 + 1],
                                OP.mult, OP.add)
                            nc.sync.dma_start(
                                yT_d[c2 * 128:(c2 + 1) * 128,
                                     rb * 512:(rb + 1) * 512], yo[:])
            if DBG_STOP in ("A", "B"):
                dsrc = qkv_sb[0] if DBG_STOP == "A" else qkv_sb[4]
                for i in range(20):
                    dq = statp.tile([128, C], FP, tag="dq", name="dq", bufs=2)
                    nc.vector.tensor_copy(dq[:], dsrc[:, i * C:(i + 1) * C])
                    nc.sync.dma_start(yT_d[0:128, i * C:(i + 1) * C], dq[:])

    nc.compile()
    return nc


_PROG = None


def _get_prog():
    global _PROG
    if _PROG is None:
        _PROG = build_program()
    return _PROG


def _host_prep(x, Wqkv, g1, b1, ab, Wproj, g2, b2, idxs):
    perm = np.empty(H, dtype=np.int64)
    for h in range(NH):
        base = h * (2 * DK + DV)
        perm[DK * h: DK * (h + 1)] = np.arange(base, base + DK)
        perm[NH * DK + DK * h: NH * DK + DK * (h + 1)] = \
            np.arange(base + DK, base + 2 * DK)
        perm[2 * NH * DK + DV * h: 2 * NH * DK + DV * (h + 1)] = \
            np.arange(base + 2 * DK, base + 2 * DK + DV)
    x = np.asarray(x, dtype=np.float32)
    Wqkv = np.asarray(Wqkv, dtype=np.float32)
    wqkvT = np.ascontiguousarray(Wqkv[perm, :].T).astype(ml_dtypes.bfloat16)
    g1c = np.ascontiguousarray(np.asarray(g1, np.float32)[perm].reshape(NHC, 128).T)
    b1c = np.ascontiguousarray(np.asarray(b1, np.float32)[perm].reshape(NHC, 128).T)
    wprojT = np.ascontiguousarray(np.asarray(Wproj, np.float32).T).astype(
        ml_dtypes.bfloat16)                                            # (1024, 256)
    E = np.exp(np.asarray(ab, np.float32))[:, np.asarray(idxs)]    # (8, 320, 320)
    eb4 = np.zeros((2, 3, 128, 4 * N), dtype=ml_dtypes.bfloat16)
    for g in range(2):
        for mc in range(3):
            ms = MCS[mc]
            mb = 64 if mc == 2 else 0
            for hh in range(4):
                eb4[g, mc, mb:mb + ms, hh * N:(hh + 1) * N] = \
                    E[4 * g + hh, 128 * mc:128 * mc + ms, :].astype(
                        ml_dtypes.bfloat16)
    common = {
        "wqkvT": wqkvT, "wprojT": wprojT, "eb4": eb4,
        "g1c": g1c, "b1c": b1c,
        "g2c": np.ascontiguousarray(
            np.asarray(g2, np.float32).reshape(2, 128).T),
        "b2c": np.ascontiguousarray(
            np.asarray(b2, np.float32).reshape(2, 128).T),
    }
    in_maps = []
    for c in range(NCORES):
        m = dict(common)
        xs = x[c * BL:(c + 1) * BL].reshape(R, C)
        m["xT"] = np.ascontiguousarray(xs.T).astype(ml_dtypes.bfloat16)
        xb = np.ones((R, C + 1), dtype=ml_dtypes.bfloat16)
        xb[:, :C] = xs.astype(ml_dtypes.bfloat16)
        m["xb"] = xb
        in_maps.append(m)
    return in_maps


def _run(in_maps, trace=False):
    nc = _get_prog()
    res = run_bass_kernel_spmd(nc, in_maps, core_ids=list(range(NCORES)),
                               trace=trace)
    out = np.concatenate(
        [np.asarray(res.results[c]["yT"]).T.reshape(BL, N, C)
         for c in range(NCORES)], axis=0)
    return out.astype(np.float32), res


def kernel(**inputs):
    out, _ = _run(_host_prep(**inputs))
    return out


def run_traced(**inputs):
    return _run(_host_prep(**inputs), trace=True)
